# revision 11
# baseline (speedup 1.0000x reference)
"""Trainium2 Bass kernel for nn_AttentionDecoder (B=64,S=256,I=H=512,C=100,E=128,T=32).

Data-parallel over batch across 8 NeuronCores (8 batch rows per core).
Per core, per decode step (31 steps):

  projPrev = h @ W_h2h.T            PE streams W_h2h.T          -> psum [8,512]
  projPrevT via PE transposes                                   -> sbuf [128,(c,b)]
  score    = projH + projPrevT      DVE tensor_scalar per (c,b)
  tanh                              ACT, [128,2048] per h-chunk
  e        = W_score . tanh         PE masked-W, b-pairs        -> psum [8,512]
  softmax (no max-subtract; exact math, |e| bounded ~40)
  alphaT via PE transposes -> masked diag tiles
  ctx      = alpha @ batch_H        PE masked-alpha             -> psum [8,512]
  gates    = xT.T @ W_catT          PE streams W (bf16, N=1024) -> psum [8,2048]
  LSTM elementwise                  ACT tanh (sigmoid via tanh) + DVE
  hT via PE transposes; logits = W_gen @ h                      -> dram out

proj_H = batch_H @ W_i2h.T + b_i2h hoisted into a prolog.  All host tensors are
pre-laid into [128, F] sbuf images so every DMA is a plain contiguous copy.
Sigmoid is computed as 0.5*tanh(0.5x)+0.5 so ACT never switches function table
sets (tanh and exp share one set).
"""

import numpy as np
import ml_dtypes

import concourse.bass as bass
import concourse.bacc as bacc
import concourse.mybir as mybir
import concourse.tile as tile
from concourse.bass_utils import run_bass_kernel_spmd

F32 = mybir.dt.float32
BF16 = mybir.dt.bfloat16
AF = mybir.ActivationFunctionType
ALU = mybir.AluOpType

B, S, I, H, C, E, T = 64, 256, 512, 512, 100, 128, 32
NCORES = 8
BL = B // NCORES          # 8 local batch rows
NSTEP = T - 1             # 31
NEG_INF = -1e30

# gate reorder: reference rows of W_ih/W_hh are [i; f; g; o]; we use [i; f; o; g]
_GATE_PERM = np.concatenate([
    np.arange(0, H), np.arange(H, 2 * H), np.arange(3 * H, 4 * H),
    np.arange(2 * H, 3 * H),
])


def _build(nsteps=NSTEP, gate_bias=False):
    nc = bacc.Bacc("TRN2", target_bir_lowering=False, debug=False,
                   num_devices=NCORES)
    NKC = 10 if gate_bias else 9          # gates lhsT chunk count

    # ---- dram inputs (per-core, host pre-laid) ----
    d_hbs = nc.dram_tensor("hbs", [128, BL * 2 * I], F32, kind="ExternalInput")
    d_hbt = nc.dram_tensor("hbt", [128, BL * 4 * S], F32, kind="ExternalInput")
    d_wi2ht = nc.dram_tensor("wi2ht", [128, 16 * 128], F32, kind="ExternalInput")
    d_wh2ht = nc.dram_tensor("wh2ht", [128, 4 * H], F32, kind="ExternalInput")
    d_wscm = nc.dram_tensor("wscm", [128, 4 * BL * BL], F32, kind="ExternalInput")
    d_wcat = nc.dram_tensor("wcat", [128, NKC * 4 * H], BF16, kind="ExternalInput")
    d_wgent = nc.dram_tensor("wgent", [128, 4 * C], F32, kind="ExternalInput")
    d_embt = nc.dram_tensor("embt", [128, nsteps * BL], BF16, kind="ExternalInput")
    d_maskb = nc.dram_tensor("maskb", [BL, S], F32, kind="ExternalInput")
    d_bh2h = nc.dram_tensor("bh2h", [128, 4], F32, kind="ExternalInput")
    d_bi2h = nc.dram_tensor("bi2h", [128, 4], F32, kind="ExternalInput")
    d_bgen = nc.dram_tensor("bgen", [128, 1], F32, kind="ExternalInput")
    d_e0 = nc.dram_tensor("e0col", [128, BL], BF16, kind="ExternalInput")
    d_ident = nc.dram_tensor("ident", [128, 128], F32, kind="ExternalInput")
    d_out = nc.dram_tensor("out", [nsteps, C, BL], F32, kind="ExternalOutput")
    out_ap = d_out.ap()

    with tile.TileContext(nc) as tc:
        with tc.tile_pool(name="cst", bufs=1) as cst, \
             tc.tile_pool(name="st", bufs=1) as st:
            # ---- persistent constants ----
            hbs = cst.tile([128, BL * 2 * I], F32)
            wh2ht = cst.tile([128, 4 * H], F32)
            wscm = cst.tile([128, 4 * BL * BL], F32)
            wcat = cst.tile([128, NKC * 4 * H], BF16)
            wgent = cst.tile([128, 4 * C], F32)
            embt = cst.tile([128, nsteps * BL], BF16)
            maskb = cst.tile([BL, S], F32)
            bh2h = cst.tile([128, 4], F32)
            bgen = cst.tile([128, 1], F32)
            e0c = cst.tile([128, BL], BF16)
            ident = cst.tile([128, 128], F32)
            for tle, dr in ((hbs, d_hbs), (wh2ht, d_wh2ht), (wscm, d_wscm),
                            (wcat, d_wcat), (wgent, d_wgent), (embt, d_embt),
                            (maskb, d_maskb), (bh2h, d_bh2h), (bgen, d_bgen),
                            (e0c, d_e0), (ident, d_ident)):
                nc.sync.dma_start(tle[:], dr.ap())

            # ---- persistent state ----
            hT = st.tile([128, 4 * BL], F32)      # h^T, cols (c,b)
            hTb = st.tile([128, 4 * BL], BF16)    # bf16 copy for gates lhsT
            c_st = st.tile([BL, H], F32)          # c state, [b, h]
            projH = st.tile([128, 4 * BL * S], F32)  # cols (c,b,s)
            pp_pad = st.tile([128, H], F32)       # zero-padded transpose inputs
            al_pad = st.tile([128, S], F32)
            hn_pad = st.tile([128, H], F32)
            ctx_pad = st.tile([128, I], F32)
            atm = st.tile([128, 2 * BL * BL], F32)  # masked alphaT diag tiles
            for z in (hT, hTb, c_st, pp_pad, al_pad, hn_pad, ctx_pad, atm):
                nc.vector.memset(z[:], 0.0)

            # ---- psum pools (8 banks: psA 2 + psT 1 + psC 1 + psG 4;
            # the prolog reuses the psG slot for its [128,2048] accumulator) ----
            with tc.tile_pool(name="psA", bufs=2, space="PSUM") as psA, \
                 tc.tile_pool(name="psT", bufs=1, space="PSUM") as psT, \
                 tc.tile_pool(name="psC", bufs=1, space="PSUM") as psC, \
                 tc.tile_pool(name="psG", bufs=1, space="PSUM") as psG:
                # ---- prolog: projH = batch_H @ W_i2h.T + b_i2h ----
                # hbt cols (ki, b, s) so b-pairs are contiguous N=512 runs
                with tc.tile_pool(name="prolog", bufs=1) as pro:
                    hbt = pro.tile([128, BL * 4 * S], F32)
                    wi2ht = pro.tile([128, 16 * 128], F32)
                    bi2h = pro.tile([128, 4], F32)
                    nc.sync.dma_start(hbt[:], d_hbt.ap())
                    nc.sync.dma_start(wi2ht[:], d_wi2ht.ap())
                    nc.sync.dma_start(bi2h[:], d_bi2h.ap())
                    for mh in range(4):
                        ph = psG.tile([128, BL * S], F32, tag="ps_g")
                        for nq in range(4):          # bank-aligned N=512 slices
                            for ki in range(4):
                                lhsT = wi2ht[:, (ki * 4 + mh) * 128:
                                             (ki * 4 + mh + 1) * 128]
                                rhs = hbt[:, ki * BL * S + nq * 512:
                                          ki * BL * S + (nq + 1) * 512]
                                nc.tensor.matmul(ph[:, nq * 512:(nq + 1) * 512],
                                                 lhsT, rhs,
                                                 start=(ki == 0), stop=(ki == 3))
                        nc.vector.tensor_scalar_add(
                            projH[:, mh * BL * S:(mh + 1) * BL * S], ph[:],
                            bi2h[:, mh:mh + 1])

                import contextlib
                _stk = contextlib.ExitStack()
                sp = _stk.enter_context(tc.tile_pool(name="step", bufs=3))
                scp = _stk.enter_context(tc.tile_pool(name="sc", bufs=2))
                thp = _stk.enter_context(tc.tile_pool(name="th", bufs=2))
                for t in range(nsteps):
                    # ---- projPrev = h @ W_h2h.T  -> [8, 512] ----
                    ps_pp = psA.tile([BL, H], F32, tag="ps_small")
                    for c1 in range(4):
                        nc.tensor.matmul(ps_pp[:], hT[:, c1 * BL:(c1 + 1) * BL],
                                         wh2ht[:, c1 * H:(c1 + 1) * H],
                                         start=(c1 == 0), stop=(c1 == 3))
                    nc.vector.tensor_copy(pp_pad[:BL, :], ps_pp[:])
                    # transpose to ppT [128, (c,b)] and add b_h2h
                    ppT = sp.tile([128, 4 * BL], F32, tag="ppT")
                    for c2 in range(4):
                        ps_tr = psT.tile([128, 128], F32, tag="ps_tr")
                        nc.tensor.transpose(ps_tr[:],
                                            pp_pad[:, c2 * 128:(c2 + 1) * 128],
                                            ident[:])
                        nc.vector.tensor_scalar_add(ppT[:, c2 * BL:(c2 + 1) * BL],
                                                    ps_tr[:, :BL],
                                                    bh2h[:, c2:c2 + 1])

                    # ---- score = tanh(projH + projPrev); e = W_score . score ----
                    ps_e = psA.tile([BL, S], F32, tag="ps_small")
                    for c in range(4):
                        sc_t = scp.tile([128, BL * S], F32, tag="sc")
                        for b in range(BL):
                            nc.vector.tensor_scalar_add(
                                sc_t[:, b * S:(b + 1) * S],
                                projH[:, (c * BL + b) * S:(c * BL + b + 1) * S],
                                ppT[:, c * BL + b:c * BL + b + 1])
                        th_t = thp.tile([128, BL * S], F32, tag="th")
                        nc.scalar.activation(th_t[:], sc_t[:], AF.Tanh)
                        for b in range(BL):
                            nc.tensor.matmul(
                                ps_e[:],
                                wscm[:, (c * BL + b) * BL:(c * BL + b + 1) * BL],
                                th_t[:, b * S:(b + 1) * S],
                                start=(c == 0 and b == 0),
                                stop=(c == 3 and b == 7))

                    # ---- softmax (no max subtraction) ----
                    e_sb = sp.tile([BL, S], F32, tag="e_sb")
                    nc.vector.tensor_tensor(e_sb[:], ps_e[:], maskb[:], ALU.add)
                    exp_sb = sp.tile([BL, S], F32, tag="exp_sb")
                    esum = sp.tile([BL, 1], F32, tag="esum")
                    nc.scalar.activation(exp_sb[:], e_sb[:], AF.Exp,
                                         accum_out=esum[:])
                    recip = sp.tile([BL, 1], F32, tag="recip")
                    nc.vector.reciprocal(recip[:], esum[:])
                    nc.vector.tensor_scalar_mul(al_pad[:BL, :], exp_sb[:], recip[:])

                    # alphaT -> masked diag tiles atm[128, (cs,b,j)], j==b col only
                    for cs in range(2):
                        ps_tr = psT.tile([128, 128], F32, tag="ps_tr")
                        nc.tensor.transpose(ps_tr[:],
                                            al_pad[:, cs * 128:(cs + 1) * 128],
                                            ident[:])
                        for b in range(BL):
                            col = (cs * BL + b) * BL + b
                            nc.vector.tensor_copy(atm[:, col:col + 1],
                                                  ps_tr[:, b:b + 1])

                    # ---- ctx [8, 512] = sum_s alpha[b,s] batch_H[b,s,:] ----
                    ps_ctx = psC.tile([BL, I], F32, tag="ps_ctx")
                    for b in range(BL):
                        for cs in range(2):
                            nc.tensor.matmul(
                                ps_ctx[:],
                                atm[:, (cs * BL + b) * BL:(cs * BL + b + 1) * BL],
                                hbs[:, (b * 2 + cs) * I:(b * 2 + cs + 1) * I],
                                start=(b == 0 and cs == 0),
                                stop=(b == 7 and cs == 1))
                    nc.vector.tensor_copy(ctx_pad[:BL, :], ps_ctx[:])
                    # transpose ctx -> xc_bf [128, (ci,b)] bf16
                    xc_bf = sp.tile([128, 4 * BL], BF16, tag="xc_bf")
                    for ci in range(4):
                        ps_tr = psT.tile([128, 128], F32, tag="ps_tr")
                        nc.tensor.transpose(ps_tr[:],
                                            ctx_pad[:, ci * 128:(ci + 1) * 128],
                                            ident[:])
                        nc.vector.tensor_copy(xc_bf[:, ci * BL:(ci + 1) * BL],
                                              ps_tr[:, :BL])

                    # ---- gates = x @ W_cat.T (+ b)  bf16 stream, N=1024 ----
                    ps_g = psG.tile([BL, 4 * H], F32, tag="ps_g")
                    lhs_chunks = ([xc_bf[:, ci * BL:(ci + 1) * BL] for ci in range(4)]
                                  + [embt[:, t * BL:(t + 1) * BL]]
                                  + [hTb[:, ch * BL:(ch + 1) * BL] for ch in range(4)])
                    if gate_bias:
                        lhs_chunks.append(e0c[:])
                    for kc, lh in enumerate(lhs_chunks):
                        for nq in range(4):
                            nc.tensor.matmul(
                                ps_g[:, nq * 512:(nq + 1) * 512], lh,
                                wcat[:, kc * 2048 + nq * 512:
                                     kc * 2048 + (nq + 1) * 512],
                                start=(kc == 0), stop=(kc == NKC - 1))

                    # ---- LSTM elementwise (gates i|f|o|g); sigmoid via tanh ----
                    sig_sb = sp.tile([BL, 3 * H], F32, tag="sig_sb")
                    nc.scalar.activation(sig_sb[:], ps_g[:, 0:3 * H], AF.Tanh,
                                         scale=0.5)
                    nc.vector.tensor_scalar(sig_sb[:], sig_sb[:], 0.5, 0.5,
                                            ALU.mult, ALU.add)
                    tg_sb = sp.tile([BL, H], F32, tag="tg_sb")
                    nc.scalar.activation(tg_sb[:], ps_g[:, 3 * H:4 * H], AF.Tanh)
                    t1 = sp.tile([BL, H], F32, tag="t1")
                    nc.vector.tensor_mul(t1[:], sig_sb[:, H:2 * H], c_st[:])
                    t2 = sp.tile([BL, H], F32, tag="t2")
                    nc.vector.tensor_mul(t2[:], sig_sb[:, 0:H], tg_sb[:])
                    nc.vector.tensor_add(c_st[:], t1[:], t2[:])
                    tc_sb = sp.tile([BL, H], F32, tag="tc_sb")
                    nc.scalar.activation(tc_sb[:], c_st[:], AF.Tanh)
                    nc.vector.tensor_mul(hn_pad[:BL, :], sig_sb[:, 2 * H:3 * H],
                                         tc_sb[:])

                    # ---- h^T update ----
                    for c2 in range(4):
                        ps_tr = psT.tile([128, 128], F32, tag="ps_tr")
                        nc.tensor.transpose(ps_tr[:],
                                            hn_pad[:, c2 * 128:(c2 + 1) * 128],
                                            ident[:])
                        nc.vector.tensor_copy(hT[:, c2 * BL:(c2 + 1) * BL],
                                              ps_tr[:, :BL])
                        nc.vector.tensor_copy(hTb[:, c2 * BL:(c2 + 1) * BL],
                                              ps_tr[:, :BL])

                    # ---- logits = W_gen @ h + b_gen -> [100, 8] -> dram ----
                    ps_l = psA.tile([C, BL], F32, tag="ps_small")
                    for ch in range(4):
                        nc.tensor.matmul(ps_l[:], wgent[:, ch * C:(ch + 1) * C],
                                         hT[:, ch * BL:(ch + 1) * BL],
                                         start=(ch == 0), stop=(ch == 3))
                    ol = sp.tile([C, BL], F32, tag="ol")
                    nc.vector.tensor_scalar_add(ol[:], ps_l[:], bgen[:C, 0:1])
                    nc.sync.dma_start(out_ap[t], ol[:])

                _stk.close()

    nc.compile()
    return nc


def _prep_inputs(batch_H, text, mask, W_i2h, b_i2h, W_h2h, b_h2h, W_score,
                 b_score, embed, W_ih, b_ih, W_hh, b_hh, W_gen, b_gen,
                 nsteps=NSTEP, gate_bias=False):
    """Host-side shard + relayout. Returns list of per-core input dicts."""
    f32 = np.float32
    batch_H = np.asarray(batch_H, f32)
    text = np.asarray(text)
    mask = np.asarray(mask)
    W_i2h = np.asarray(W_i2h, f32); b_i2h = np.asarray(b_i2h, f32)
    W_h2h = np.asarray(W_h2h, f32); b_h2h = np.asarray(b_h2h, f32)
    W_score = np.asarray(W_score, f32).reshape(H); b_score = np.asarray(b_score, f32)
    embed = np.asarray(embed, f32)
    W_ih = np.asarray(W_ih, f32); b_ih = np.asarray(b_ih, f32)
    W_hh = np.asarray(W_hh, f32); b_hh = np.asarray(b_hh, f32)
    W_gen = np.asarray(W_gen, f32); b_gen = np.asarray(b_gen, f32)
    NKC = 10 if gate_bias else 9

    # wi2ht[p, (ki, mh, q)] = W_i2h[mh*128+q, ki*128+p]
    wi2ht = np.ascontiguousarray(
        W_i2h.reshape(4, 128, 4, 128).transpose(3, 2, 0, 1).reshape(128, 2048))
    # wh2ht[p, (c1, h2)] = W_h2h[h2, c1*128+p]
    wh2ht = np.ascontiguousarray(
        W_h2h.reshape(H, 4, 128).transpose(2, 1, 0).reshape(128, 4 * H))
    # masked W_score tiles: tile (c,b) col b holds W_c chunk, rest zero
    ws = W_score.reshape(4, 128)
    wscm = np.zeros((128, 4, BL, BL), f32)
    for c in range(4):
        for b in range(BL):
            wscm[:, c, b, b] = ws[c]
    wscm = np.ascontiguousarray(wscm.reshape(128, 4 * BL * BL))
    # gates weights, rows reordered [i|f|o|g], cols [ctx | emb | h]; opt bias row
    W_full = np.concatenate([W_ih, W_hh], axis=1)[_GATE_PERM]      # [2048, 1152]
    b_cat = (b_ih + b_hh)[_GATE_PERM]                              # [2048]
    wcat = np.zeros((128, NKC, 4 * H), f32)
    wcat[:, :9, :] = W_full.T.reshape(9, 128, 4 * H).transpose(1, 0, 2)
    if gate_bias:
        wcat[0, 9, :] = b_cat
    wcat = np.ascontiguousarray(wcat.reshape(128, NKC * 4 * H)).astype(
        ml_dtypes.bfloat16)
    # wgent[p, (ch, c)] = W_gen[c, ch*128+p]
    wgent = np.ascontiguousarray(
        W_gen.reshape(C, 4, 128).transpose(2, 1, 0).reshape(128, 4 * C))
    bh2h_t = np.ascontiguousarray(b_h2h.reshape(4, 128).T)
    bi2h_t = np.ascontiguousarray(b_i2h.reshape(4, 128).T)
    bgen_t = np.zeros((128, 1), f32)
    bgen_t[:C, 0] = b_gen
    e0c = np.zeros((128, BL), f32)
    e0c[0, :] = 1.0
    e0c = e0c.astype(ml_dtypes.bfloat16)
    ident = np.eye(128, dtype=f32)

    in_maps = []
    for k in range(NCORES):
        sl = slice(k * BL, (k + 1) * BL)
        bh = batch_H[sl]                                           # [8, 256, 512]
        hbs = np.ascontiguousarray(
            bh.reshape(BL, 2, 128, I).transpose(2, 0, 1, 3).reshape(128, BL * 2 * I))
        # hbt cols (ki, b, s)
        hbt = np.ascontiguousarray(
            bh.reshape(BL, S, 4, 128).transpose(3, 2, 0, 1).reshape(128, BL * 4 * S))
        # embt[p, (t, b)] = embed[text[b, t], p]
        emb_g = embed[text[sl, :nsteps]]                           # [8, nsteps, 128]
        embt = np.ascontiguousarray(emb_g.transpose(2, 1, 0).reshape(128, nsteps * BL)
                                    ).astype(ml_dtypes.bfloat16)
        maskb = np.where(mask[sl], 0.0, NEG_INF).astype(f32) + b_score[0]
        in_maps.append({
            "hbs": hbs, "hbt": hbt, "wi2ht": wi2ht, "wh2ht": wh2ht,
            "wscm": wscm, "wcat": wcat, "wgent": wgent, "embt": embt,
            "maskb": maskb, "bh2h": bh2h_t, "bi2h": bi2h_t, "bgen": bgen_t,
            "e0col": e0c, "ident": ident,
        })
    return in_maps


_NC_CACHE = {}


def kernel(**inputs):
    nsteps = NSTEP
    if "build_nsteps" in inputs:          # dev hook
        nsteps = inputs.pop("build_nsteps")
    gate_bias = bool(np.any(np.asarray(inputs["b_ih"]))
                     or np.any(np.asarray(inputs["b_hh"])))
    key = (nsteps, gate_bias)
    if key not in _NC_CACHE:
        _NC_CACHE[key] = _build(nsteps, gate_bias)
    nc = _NC_CACHE[key]
    in_maps = _prep_inputs(nsteps=nsteps, gate_bias=gate_bias, **inputs)
    res = run_bass_kernel_spmd(nc, in_maps, core_ids=list(range(NCORES)))
    outs = [res.results[k]["out"].transpose(2, 0, 1) for k in range(NCORES)]
    outputs = np.concatenate(outs, axis=0)                          # [64, ns, 100]
    targets = np.asarray(inputs["text"])[:, 1:]
    return outputs, targets


# revision 23
# speedup vs baseline: 1.6448x; 1.6448x over previous
"""Trainium2 Bass kernel for nn_AttentionDecoder (B=64,S=256,I=H=512,C=100,E=128,T=32).

Data-parallel over batch across 8 NeuronCores (8 batch rows per core).
Per core, per decode step (31 steps):

  projPrevT = (W_h2h @ h)^T         PE, W_h2h^T stationary      -> psum [128,(c,b)]
  score     = projH + projPrevT     DVE tensor_scalar (bf16 4x)
  tanh                              ACT, [128,2048] per h-chunk
  e         = W_score . tanh        PE masked-W (bf16)          -> psum [8,256]
  softmax (no max-subtract; exact math, |e| bounded ~40)
  alphaT via PE transposes -> masked diag tiles (bf16)
  ctx       = alpha @ batch_H       PE masked-alpha (bf16)      -> psum [8,512]
  gates     = xT.T @ W_catT         PE streams W (bf16), one psum
                                    tile per gate, order g|i|f|o
  LSTM elementwise                  ACT tanh only (sigmoid via tanh) + DVE
  hT via PE transposes; logits = W_gen @ h (bf16)               -> dram out

proj_H = batch_H @ W_i2h.T + b_i2h is hoisted into a prolog.  All host-side
tensors are pre-laid into [128, F] sbuf images so every DMA is contiguous.
Sigmoid is 0.5*tanh(0.5x)+0.5 so ACT never switches function-table sets.
All PE streams are bf16 (fp32 matmul runs at 1/4 rate); accumulation and the
softmax/LSTM state stay fp32.
"""

import numpy as np
import ml_dtypes

import concourse.bass as bass
import concourse.bacc as bacc
import concourse.mybir as mybir
import concourse.tile as tile
from concourse.bass_utils import run_bass_kernel_spmd

F32 = mybir.dt.float32
BF16 = mybir.dt.bfloat16
AF = mybir.ActivationFunctionType
ALU = mybir.AluOpType

B, S, I, H, C, E, T = 64, 256, 512, 512, 100, 128, 32
NCORES = 8
BL = B // NCORES          # 8 local batch rows
NSTEP = T - 1             # 31
NEG_INF = -1e30

# gate layout [g | i | f | o]; reference rows of W_ih/W_hh are [i; f; g; o]
_GATE_PERM = np.concatenate([
    np.arange(2 * H, 3 * H), np.arange(0, H), np.arange(H, 2 * H),
    np.arange(3 * H, 4 * H),
])


def _build(nsteps=NSTEP, gate_bias=False, use_maskbias=False, repeat=1):
    nc = bacc.Bacc("TRN2", target_bir_lowering=False, debug=False,
                   num_devices=NCORES)
    NKC = 10 if gate_bias else 9          # gates lhsT chunk count

    # ---- dram inputs (per-core, host pre-laid) ----
    d_hbs = nc.dram_tensor("hbs", [128, BL * 2 * I], BF16, kind="ExternalInput")
    d_hbt = nc.dram_tensor("hbt", [128, BL * 4 * S], BF16, kind="ExternalInput")
    d_wi2ht = nc.dram_tensor("wi2ht", [128, 16 * 128], BF16, kind="ExternalInput")
    d_wh2hs = nc.dram_tensor("wh2hs", [128, 16 * 128], BF16, kind="ExternalInput")
    d_wscm = nc.dram_tensor("wscm", [128, 4 * BL * BL], BF16, kind="ExternalInput")
    d_wcat = nc.dram_tensor("wcat", [128, NKC * 4 * H], BF16, kind="ExternalInput")
    d_wgent = nc.dram_tensor("wgent", [128, 4 * C], BF16, kind="ExternalInput")
    d_embt = nc.dram_tensor("embt", [128, nsteps * BL], BF16, kind="ExternalInput")
    d_maskb = nc.dram_tensor("maskb", [BL, S], F32, kind="ExternalInput")
    d_bh2h = nc.dram_tensor("bh2h", [128, 4], F32, kind="ExternalInput")
    d_bi2h = nc.dram_tensor("bi2h", [128, 4], F32, kind="ExternalInput")
    d_bgen = nc.dram_tensor("bgen", [128, 1], F32, kind="ExternalInput")
    d_e0 = nc.dram_tensor("e0col", [128, BL], BF16, kind="ExternalInput")
    d_ident = nc.dram_tensor("ident", [128, 128], F32, kind="ExternalInput")
    d_identb = nc.dram_tensor("identb", [128, 128], BF16, kind="ExternalInput")
    d_out = nc.dram_tensor("out", [nsteps, C, BL], F32, kind="ExternalOutput")
    out_ap = d_out.ap()

    with tile.TileContext(nc) as tc:
        with tc.tile_pool(name="cst", bufs=1) as cst, \
             tc.tile_pool(name="st", bufs=1) as st:
            # ---- persistent constants ----
            hbs = cst.tile([128, BL * 2 * I], BF16)
            wh2hs = cst.tile([128, 16 * 128], BF16)
            wscm = cst.tile([128, 4 * BL * BL], BF16)
            wcat = cst.tile([128, NKC * 4 * H], BF16)
            wgent = cst.tile([128, 4 * C], BF16)
            embt = cst.tile([128, nsteps * BL], BF16)
            maskb = cst.tile([BL, S], F32)
            bh2h = cst.tile([128, 4], F32)
            bgen = cst.tile([128, 1], F32)
            e0c = cst.tile([128, BL], BF16)
            ident = cst.tile([128, 128], F32)
            identb = cst.tile([128, 128], BF16)
            # balance the big constants across the three DMA queue engines
            for eng, tle, dr in ((nc.sync, hbs, d_hbs),
                                 (nc.gpsimd, wcat, d_wcat),
                                 (nc.scalar, wh2hs, d_wh2hs),
                                 (nc.scalar, wscm, d_wscm),
                                 (nc.scalar, wgent, d_wgent),
                                 (nc.scalar, embt, d_embt),
                                 (nc.sync, maskb, d_maskb),
                                 (nc.sync, bh2h, d_bh2h),
                                 (nc.sync, bgen, d_bgen),
                                 (nc.sync, e0c, d_e0),
                                 (nc.sync, ident, d_ident),
                                 (nc.sync, identb, d_identb)):
                eng.dma_start(tle[:], dr.ap())

            # ---- persistent state ----
            hT = st.tile([128, 4 * BL], BF16)     # h^T, cols (c,b)
            c_st = st.tile([BL, H], F32)          # c state, [b, h]
            projH = st.tile([128, 4 * BL * S], BF16)  # cols (c,b,s)
            al_pad = st.tile([128, S], BF16)      # zero-padded transpose inputs
            hn_pad = st.tile([128, H], BF16)
            ctx_pad = st.tile([128, I], BF16)
            atm = st.tile([128, 2 * BL * BL], BF16)  # masked alphaT diag tiles
            for z in (al_pad, hn_pad, ctx_pad, atm):
                nc.vector.memset(z[:], 0.0)
            if repeat == 1:
                for z in (hT, c_st):
                    nc.vector.memset(z[:], 0.0)

            # ---- psum pools (8 banks: psA 2 + psT 1 + psC 1 + psG 4) ----
            with tc.tile_pool(name="psA", bufs=2, space="PSUM") as psA, \
                 tc.tile_pool(name="psT", bufs=1, space="PSUM") as psT, \
                 tc.tile_pool(name="psC", bufs=1, space="PSUM") as psC, \
                 tc.tile_pool(name="psG", bufs=4, space="PSUM") as psG:
                # ---- prolog: projH = batch_H @ W_i2h.T + b_i2h ----
                # hbt cols (ki, b, s) so (b,s) slices are contiguous N=512 runs
                with tc.tile_pool(name="prolog", bufs=1) as pro:
                    hbt = pro.tile([128, BL * 4 * S], BF16)
                    wi2ht = pro.tile([128, 16 * 128], BF16)
                    bi2h = pro.tile([128, 4], F32)
                    nc.gpsimd.dma_start(hbt[:], d_hbt.ap())
                    nc.sync.dma_start(wi2ht[:], d_wi2ht.ap())
                    nc.sync.dma_start(bi2h[:], d_bi2h.ap())
                    for mh in range(4):
                        for nq in range(4):          # 512-wide (b,s) slices
                            ph = psG.tile([128, 512], F32, tag="ps_g")
                            for ki in range(4):
                                lhsT = wi2ht[:, (ki * 4 + mh) * 128:
                                             (ki * 4 + mh + 1) * 128]
                                rhs = hbt[:, ki * BL * S + nq * 512:
                                          ki * BL * S + (nq + 1) * 512]
                                nc.tensor.matmul(ph[:], lhsT, rhs,
                                                 start=(ki == 0), stop=(ki == 3))
                            nc.vector.tensor_scalar_add(
                                projH[:, mh * BL * S + nq * 512:
                                      mh * BL * S + (nq + 1) * 512],
                                ph[:], bi2h[:, mh:mh + 1])

                import contextlib
                _stk = contextlib.ExitStack()
                sp = _stk.enter_context(tc.tile_pool(name="step", bufs=3))
                scp = _stk.enter_context(tc.tile_pool(name="sc", bufs=2))
                thp = _stk.enter_context(tc.tile_pool(name="th", bufs=2))
                if repeat > 1:      # timing builds: repeat the whole decode
                    _loop = _stk.enter_context(tc.For_i(0, repeat, 1))
                    for z in (hT, c_st):
                        nc.vector.memset(z[:], 0.0)
                for t in range(nsteps):
                    # ---- projPrevT[h2,(c,b)] = (W_h2h @ h)^T, stationary ----
                    ps_ppT = psA.tile([128, 4 * BL], F32, tag="ps_small")
                    for mh2 in range(4):
                        for k1 in range(4):
                            lhsT = wh2hs[:, (k1 * 4 + mh2) * 128:
                                         (k1 * 4 + mh2 + 1) * 128]
                            nc.tensor.matmul(
                                ps_ppT[:, mh2 * BL:(mh2 + 1) * BL], lhsT,
                                hT[:, k1 * BL:(k1 + 1) * BL],
                                start=(k1 == 0), stop=(k1 == 3))

                    # ---- gates part 1: emb/h chunks (PE fills tanh phase) ----
                    # one psum tile per gate, column order g|i|f|o
                    g_ps = [psG.tile([BL, H], F32, tag="ps_g", name=f"g_ps{_g}")
                            for _g in range(4)]
                    emb_h_chunks = ([(4, embt[:, t * BL:(t + 1) * BL])]
                                    + [(5 + ch, hT[:, ch * BL:(ch + 1) * BL])
                                       for ch in range(4)])
                    for nq in range(4):
                        for j, (kc, lh) in enumerate(emb_h_chunks):
                            nc.tensor.matmul(
                                g_ps[nq][:], lh,
                                wcat[:, kc * 2048 + nq * 512:
                                     kc * 2048 + (nq + 1) * 512],
                                start=(j == 0), stop=False)

                    # ---- score = tanh(projH + projPrev); e = W_score . score ----
                    ps_e = psA.tile([BL, S], F32, tag="ps_small")
                    for c in range(4):
                        sc_t = scp.tile([128, BL * S], BF16, tag="sc")
                        for b in range(BL):
                            nc.vector.tensor_scalar(
                                sc_t[:, b * S:(b + 1) * S],
                                projH[:, (c * BL + b) * S:(c * BL + b + 1) * S],
                                ps_ppT[:, c * BL + b:c * BL + b + 1],
                                bh2h[:, c:c + 1], ALU.add, ALU.add)
                        th_t = thp.tile([128, BL * S], BF16, tag="th")
                        nc.scalar.activation(th_t[:], sc_t[:], AF.Tanh)
                        for b in range(BL):
                            nc.tensor.matmul(
                                ps_e[:],
                                wscm[:, (c * BL + b) * BL:(c * BL + b + 1) * BL],
                                th_t[:, b * S:(b + 1) * S],
                                start=(c == 0 and b == 0),
                                stop=(c == 3 and b == 7))

                    # ---- softmax (no max subtraction); exp into al_pad,
                    # the 1/sum normalization is applied to ctx rows below ----
                    esum = sp.tile([BL, 1], F32, tag="esum")
                    if use_maskbias:
                        e_sb = sp.tile([BL, S], F32, tag="e_sb")
                        nc.vector.tensor_tensor(e_sb[:], ps_e[:], maskb[:],
                                                ALU.add)
                        nc.scalar.activation(al_pad[:BL, :], e_sb[:], AF.Exp,
                                             accum_out=esum[:])
                    else:
                        nc.scalar.activation(al_pad[:BL, :], ps_e[:], AF.Exp,
                                             accum_out=esum[:])
                    recip = sp.tile([BL, 1], F32, tag="recip")
                    nc.vector.reciprocal(recip[:], esum[:])

                    # alphaT -> masked diag tiles atm[128, (cs,b,j)], col j==b
                    for cs in range(2):
                        ps_tr = psT.tile([128, 128], BF16, tag="ps_tr")
                        nc.tensor.transpose(ps_tr[:],
                                            al_pad[:, cs * 128:(cs + 1) * 128],
                                            identb[:])
                        for b in range(BL):
                            col = (cs * BL + b) * BL + b
                            nc.vector.tensor_copy(atm[:, col:col + 1],
                                                  ps_tr[:, b:b + 1])

                    # ---- ctx [8, 512] = sum_s alpha[b,s] batch_H[b,s,:] ----
                    ps_ctx = psC.tile([BL, I], F32, tag="ps_ctx")
                    for b in range(BL):
                        for cs in range(2):
                            nc.tensor.matmul(
                                ps_ctx[:],
                                atm[:, (cs * BL + b) * BL:(cs * BL + b + 1) * BL],
                                hbs[:, (b * 2 + cs) * I:(b * 2 + cs + 1) * I],
                                start=(b == 0 and cs == 0),
                                stop=(b == 7 and cs == 1))
                    nc.vector.tensor_scalar_mul(ctx_pad[:BL, :], ps_ctx[:],
                                                recip[:])
                    # transpose ctx -> xc_bf [128, (ci,b)] bf16
                    xc_bf = sp.tile([128, 4 * BL], BF16, tag="xc_bf")
                    for ci in range(4):
                        ps_tr = psT.tile([128, 128], BF16, tag="ps_tr")
                        nc.tensor.transpose(ps_tr[:],
                                            ctx_pad[:, ci * 128:(ci + 1) * 128],
                                            identb[:])
                        nc.vector.tensor_copy(xc_bf[:, ci * BL:(ci + 1) * BL],
                                              ps_tr[:, :BL])

                    # ---- gates part 2: ctx chunks, then activations ----
                    ctx_chunks = [(ci, xc_bf[:, ci * BL:(ci + 1) * BL])
                                  for ci in range(4)]
                    if gate_bias:
                        ctx_chunks.append((9, e0c[:]))
                    sig_sb = sp.tile([BL, 3 * H], F32, tag="sig_sb")
                    tg_sb = sp.tile([BL, H], F32, tag="tg_sb")
                    tc_sb = sp.tile([BL, H], F32, tag="tc_sb")
                    t1 = sp.tile([BL, H], F32, tag="t1")
                    t2 = sp.tile([BL, H], F32, tag="t2")
                    for nq in range(4):              # gate order g, i, f, o
                        for j, (kc, lh) in enumerate(ctx_chunks):
                            nc.tensor.matmul(
                                g_ps[nq][:], lh,
                                wcat[:, kc * 2048 + nq * 512:
                                     kc * 2048 + (nq + 1) * 512],
                                start=False, stop=(j == len(ctx_chunks) - 1))
                        if nq == 0:                  # g: plain tanh
                            nc.scalar.activation(tg_sb[:], g_ps[nq][:], AF.Tanh)
                        else:                        # i/f/o: sigmoid via tanh
                            gs = sig_sb[:, (nq - 1) * H:nq * H]
                            nc.scalar.activation(gs, g_ps[nq][:], AF.Tanh,
                                                 scale=0.5)
                            nc.vector.tensor_scalar(gs, gs, 0.5, 0.5,
                                                    ALU.mult, ALU.add)
                            if nq == 1:              # t2 = sig(i)*tanh(g)
                                nc.vector.tensor_mul(t2[:], gs, tg_sb[:])
                            elif nq == 2:            # c = sig(f)*c + t2
                                nc.vector.tensor_mul(t1[:], gs, c_st[:])
                                nc.vector.tensor_add(c_st[:], t1[:], t2[:])
                                nc.scalar.activation(tc_sb[:], c_st[:], AF.Tanh)
                    nc.vector.tensor_mul(hn_pad[:BL, :], sig_sb[:, 2 * H:3 * H],
                                         tc_sb[:])

                    # ---- h^T update ----
                    for c2 in range(4):
                        ps_tr = psT.tile([128, 128], BF16, tag="ps_tr")
                        nc.tensor.transpose(ps_tr[:],
                                            hn_pad[:, c2 * 128:(c2 + 1) * 128],
                                            identb[:])
                        nc.vector.tensor_copy(hT[:, c2 * BL:(c2 + 1) * BL],
                                              ps_tr[:, :BL])

                    # ---- logits = W_gen @ h + b_gen -> [100, 8] -> dram ----
                    ps_l = psA.tile([C, BL], F32, tag="ps_small")
                    for ch in range(4):
                        nc.tensor.matmul(ps_l[:], wgent[:, ch * C:(ch + 1) * C],
                                         hT[:, ch * BL:(ch + 1) * BL],
                                         start=(ch == 0), stop=(ch == 3))
                    ol = sp.tile([C, BL], F32, tag="ol")
                    nc.vector.tensor_scalar_add(ol[:], ps_l[:], bgen[:C, 0:1])
                    nc.sync.dma_start(out_ap[t], ol[:])
                _stk.close()

    nc.compile()
    return nc


def _prep_inputs(batch_H, text, mask, W_i2h, b_i2h, W_h2h, b_h2h, W_score,
                 b_score, embed, W_ih, b_ih, W_hh, b_hh, W_gen, b_gen,
                 nsteps=NSTEP, gate_bias=False):
    """Host-side shard + relayout. Returns list of per-core input dicts."""
    f32 = np.float32
    bf16 = ml_dtypes.bfloat16
    batch_H = np.asarray(batch_H, f32)
    text = np.asarray(text)
    mask = np.asarray(mask)
    W_i2h = np.asarray(W_i2h, f32); b_i2h = np.asarray(b_i2h, f32)
    W_h2h = np.asarray(W_h2h, f32); b_h2h = np.asarray(b_h2h, f32)
    W_score = np.asarray(W_score, f32).reshape(H); b_score = np.asarray(b_score, f32)
    embed = np.asarray(embed, f32)
    W_ih = np.asarray(W_ih, f32); b_ih = np.asarray(b_ih, f32)
    W_hh = np.asarray(W_hh, f32); b_hh = np.asarray(b_hh, f32)
    W_gen = np.asarray(W_gen, f32); b_gen = np.asarray(b_gen, f32)
    NKC = 10 if gate_bias else 9

    # wi2ht[p, (ki, mh, q)] = W_i2h[mh*128+q, ki*128+p]
    wi2ht = np.ascontiguousarray(
        W_i2h.reshape(4, 128, 4, 128).transpose(3, 2, 0, 1).reshape(128, 2048)
        ).astype(bf16)
    # wh2hs[p, (k1, mh2, q)] = W_h2h[mh2*128+q, k1*128+p]
    wh2hs = np.ascontiguousarray(
        W_h2h.reshape(4, 128, 4, 128).transpose(3, 2, 0, 1).reshape(128, 2048)
        ).astype(bf16)
    # masked W_score tiles: tile (c,b) col b holds W_c chunk, rest zero
    ws = W_score.reshape(4, 128)
    wscm = np.zeros((128, 4, BL, BL), f32)
    for c in range(4):
        for b in range(BL):
            wscm[:, c, b, b] = ws[c]
    wscm = np.ascontiguousarray(wscm.reshape(128, 4 * BL * BL)).astype(bf16)
    # gates weights, rows reordered [g|i|f|o], cols [ctx | emb | h]; opt bias
    W_full = np.concatenate([W_ih, W_hh], axis=1)[_GATE_PERM]      # [2048, 1152]
    b_cat = (b_ih + b_hh)[_GATE_PERM]                              # [2048]
    wcat = np.zeros((128, NKC, 4 * H), f32)
    wcat[:, :9, :] = W_full.T.reshape(9, 128, 4 * H).transpose(1, 0, 2)
    if gate_bias:
        wcat[0, 9, :] = b_cat
    wcat = np.ascontiguousarray(wcat.reshape(128, NKC * 4 * H)).astype(bf16)
    # wgent[p, (ch, c)] = W_gen[c, ch*128+p]
    wgent = np.ascontiguousarray(
        W_gen.reshape(C, 4, 128).transpose(2, 1, 0).reshape(128, 4 * C)
        ).astype(bf16)
    bh2h_t = np.ascontiguousarray(b_h2h.reshape(4, 128).T)
    bi2h_t = np.ascontiguousarray(b_i2h.reshape(4, 128).T)
    bgen_t = np.zeros((128, 1), f32)
    bgen_t[:C, 0] = b_gen
    e0c = np.zeros((128, BL), f32)
    e0c[0, :] = 1.0
    e0c = e0c.astype(bf16)
    ident = np.eye(128, dtype=f32)
    identb = np.eye(128, dtype=f32).astype(bf16)

    in_maps = []
    for k in range(NCORES):
        sl = slice(k * BL, (k + 1) * BL)
        bh = batch_H[sl]                                           # [8, 256, 512]
        hbs = np.ascontiguousarray(
            bh.reshape(BL, 2, 128, I).transpose(2, 0, 1, 3).reshape(128, BL * 2 * I)
            ).astype(bf16)
        # hbt cols (ki, b, s)
        hbt = np.ascontiguousarray(
            bh.reshape(BL, S, 4, 128).transpose(3, 2, 0, 1).reshape(128, BL * 4 * S)
            ).astype(bf16)
        # embt[p, (t, b)] = embed[text[b, t], p]
        emb_g = embed[text[sl, :nsteps]]                           # [8, nsteps, 128]
        embt = np.ascontiguousarray(
            emb_g.transpose(2, 1, 0).reshape(128, nsteps * BL)).astype(bf16)
        maskb = np.where(mask[sl], 0.0, NEG_INF).astype(f32) + b_score[0]
        in_maps.append({
            "hbs": hbs, "hbt": hbt, "wi2ht": wi2ht, "wh2hs": wh2hs,
            "wscm": wscm, "wcat": wcat, "wgent": wgent, "embt": embt,
            "maskb": maskb, "bh2h": bh2h_t, "bi2h": bi2h_t, "bgen": bgen_t,
            "e0col": e0c, "ident": ident, "identb": identb,
        })
    return in_maps


_NC_CACHE = {}


def kernel(**inputs):
    nsteps = NSTEP
    if "build_nsteps" in inputs:          # dev hook
        nsteps = inputs.pop("build_nsteps")
    gate_bias = bool(np.any(np.asarray(inputs["b_ih"]))
                     or np.any(np.asarray(inputs["b_hh"])))
    use_maskbias = bool((~np.asarray(inputs["mask"])).any()
                        or np.any(np.asarray(inputs["b_score"])))
    key = (nsteps, gate_bias, use_maskbias)
    if key not in _NC_CACHE:
        _NC_CACHE[key] = _build(nsteps, gate_bias, use_maskbias)
    nc = _NC_CACHE[key]
    in_maps = _prep_inputs(nsteps=nsteps, gate_bias=gate_bias, **inputs)
    res = run_bass_kernel_spmd(nc, in_maps, core_ids=list(range(NCORES)))
    outs = [res.results[k]["out"].transpose(2, 0, 1) for k in range(NCORES)]
    outputs = np.concatenate(outs, axis=0)                          # [64, ns, 100]
    targets = np.asarray(inputs["text"])[:, 1:]
    return outputs, targets
